# revision 44
# baseline (speedup 1.0000x reference)
"""MoE MLP (top-2 of 8 experts) Trainium2 kernel — fp8 DoubleRow edition.

Strategy: expert-parallel across the 8 NeuronCores (host computes exact top-2
gating, gathers each expert's tokens; core e runs expert e's two matmuls).

The two big matmuls run on the PE in fp8-e4m3 DoubleRow mode (2 contraction
subtiles per instruction at 0.5 cycles/row = 4x the fp16 MAC rate). Plain fp8
is too coarse (~5e-2 rel err), so every operand is split into hi + lo fp8
parts at a SHARED power-of-2 scale (the residual lands partly in fp8
denormals, which the PE honors) and each 256-wide contraction group runs
three DoubleRow matmuls accumulating into one PSUM group:

    x@W ~= xh@Wh + xl@Wh + xh@Wl        (the lo*lo term is negligible)

mm2 additionally skips the a-residual (al@Wh) correction on 4 of its 16
contraction groups, trading measured rel err up to ~1.4e-2 (vs the 2e-2
budget) for 4 fewer matmuls per output tile. Net matmul cycle count is
0.72x fp16 -> ~1.4x over the fp16 PE roofline.

Per-core dataflow (contraction dim on SBUF partitions throughout):
  mm1: psum[h_blk(128), tok(S)] += 3-term DR over wfc{h,l}[d,2,h] x{h,l}[d,2,S]
  act: t = prelu(psum * 2^-14, 0.5) in fp16 (ScalarE)   [t = sqrt(sa)*prelu(h)]
       t2 = t*t fp16 (VectorE)                          [t2 = sa*a]
       ah = cast_fp8(t2) (ScalarE); al = t2 - ah -> fp8 (VectorE)
  mm2: psum[tok(128), d(512)] += 3-term DR over a{h,l}[h,2,tok] wproj{h,l}[h,2,d]
  evict: out = psum * g'[token]  (g' = gate/(sa*sp), per-partition scale)

Host scatters per-expert rows back and sums (each token in exactly 2 lists).
"""

import numpy as np
from contextlib import ExitStack

B, T, D, H, E = 4, 2048, 1024, 4096, 8
N = B * T
P = 128
CHUNK = 512

# power-of-2 quantization scales (shared between hi and lo parts)
SX = 32.0        # x:    |x|max ~5.3  -> 169 < 240
SW = 1024.0      # Wfc:  |W|max ~0.11 -> 108
SA = 4.0         # a:    |a|max ~15   -> 58
SP = 1024.0      # Wproj
S0 = float(np.sqrt(SA) / (SX * SW))   # prelu pre-scale = 2/(32*1024) = 2^-14

_NC_CACHE = {}


def _build_nc(C):
    """Build + compile the per-core Bass program for capacity C tokens."""
    if C in _NC_CACHE:
        return _NC_CACHE[C]
    import concourse.bacc as bacc
    import concourse.tile as tile
    import concourse.mybir as mybir

    assert C % P == 0
    f8 = mybir.dt.float8e4
    f16 = mybir.dt.float16
    f32 = mybir.dt.float32
    AF = mybir.ActivationFunctionType
    DR = mybir.MatmulPerfMode.DoubleRow

    WS = 512             # wfc DRAM H-tile width: keeps DMA runs >= 512B
    NSL = H // WS        # 8 slices

    nc = bacc.Bacc(None, target_bir_lowering=False, debug=False)
    xh_d = nc.dram_tensor("xh", [D, C], f8, kind="ExternalInput")
    xl_d = nc.dram_tensor("xl", [D, C], f8, kind="ExternalInput")
    # wfc stored H-slice-major so each [D, 512] slice is 512B-contiguous
    wfch_d = nc.dram_tensor("wfch", [NSL, D, WS], f8, kind="ExternalInput")
    wfcl_d = nc.dram_tensor("wfcl", [NSL, D, WS], f8, kind="ExternalInput")
    wph_d = nc.dram_tensor("wph", [H, D], f8, kind="ExternalInput")
    wpl_d = nc.dram_tensor("wpl", [H, D], f8, kind="ExternalInput")
    g = nc.dram_tensor("g", [P, C // P], f32, kind="ExternalInput")
    out = nc.dram_tensor("outp", [C, D], f32, kind="ExternalOutput")

    xh_v = xh_d.ap().rearrange("(ko p) c -> p ko c", p=P)      # [128, 8, C]
    xl_v = xl_d.ap().rearrange("(ko p) c -> p ko c", p=P)
    wfch_v = wfch_d.ap().rearrange("s (ko p) j -> p s ko j", p=P)  # [128, 8, 8, 512]
    wfcl_v = wfcl_d.ap().rearrange("s (ko p) j -> p s ko j", p=P)
    wph_v = wph_d.ap().rearrange("(ko p) d -> p ko d", p=P)    # [128, 32, D]
    wpl_v = wpl_d.ap().rearrange("(ko p) d -> p ko d", p=P)
    out_v = out.ap().rearrange("(c p) d -> p c d", p=P)        # [128, C//128, D]

    KD = D // P          # 8  k-subtiles for mm1 (4 DoubleRow groups)
    KH = H // P          # 32 k-subtiles for mm2 (16 DoubleRow groups)
    DN = D // CHUNK      # 2 output-column blocks

    # Tail chunk goes SECOND: chunk 0 must be full-size (its mm1 is the DMA
    # window for the wproj load), and ending on a full chunk hides the tail's
    # short act-chain lag and shrinks the end-of-kernel drain.
    chunks = [CHUNK] * (C // CHUNK)
    if C % CHUNK:
        chunks.insert(1, C % CHUNK)

    with tile.TileContext(nc) as tc:
        with ExitStack() as ctx:
            const = ctx.enter_context(tc.tile_pool(name="const", bufs=1))
            xpool = ctx.enter_context(tc.tile_pool(name="xp", bufs=2))
            apool = ctx.enter_context(tc.tile_pool(name="apool", bufs=1))
            tpool = ctx.enter_context(tc.tile_pool(name="tp", bufs=3))
            t2pool = ctx.enter_context(tc.tile_pool(name="t2p", bufs=3))
            opool = ctx.enter_context(tc.tile_pool(name="op", bufs=4))
            ps1pool = ctx.enter_context(tc.tile_pool(name="ps1", bufs=4, space="PSUM"))
            ps2pool = ctx.enter_context(tc.tile_pool(name="ps2", bufs=4, space="PSUM"))

            # Startup-critical DMAs first. mm1 group mh consumes wfc H-slice
            # mh//4; slice 0 + x chunk0 gate the first matmuls.
            x_tiles = {}
            xht = xpool.tile([P, KD, chunks[0]], f8, tag="xh", name="xh0")
            xlt = xpool.tile([P, KD, chunks[0]], f8, tag="xl", name="xl0")
            x_tiles[0] = (xht, xlt)
            wfch_sb = const.tile([P, KD, H], f8)
            wfcl_sb = const.tile([P, KD, H], f8)

            def wfc_slice(si):
                ws = slice(si * WS, (si + 1) * WS)
                nc.sync.dma_start(wfch_sb[:, :, ws], wfch_v[:, si, :, :])
                nc.sync.dma_start(wfcl_sb[:, :, ws], wfcl_v[:, si, :, :])

            ws0 = slice(0, WS)
            nc.sync.dma_start(wfch_sb[:, :, ws0], wfch_v[:, 0, :, :])
            nc.sync.dma_start(xht[:, 0:4, :], xh_v[:, 0:4, 0:chunks[0]])
            nc.sync.dma_start(xht[:, 4:KD, :], xh_v[:, 4:KD, 0:chunks[0]])
            nc.sync.dma_start(xlt[:], xl_v[:, :, 0:chunks[0]])
            nc.sync.dma_start(wfcl_sb[:, :, ws0], wfcl_v[:, 0, :, :])
            for si in range(1, NSL):
                wfc_slice(si)
            # wproj needed only when mm2 of chunk 0 starts (~48us in), and
            # with dn as the outer mm2 loop the second D-half isn't touched
            # until ~20us later still — load half-D at a time.
            wph_sb = const.tile([P, KH, D], f8)
            wpl_sb = const.tile([P, KH, D], f8)
            for dh in range(DN):
                dsl = slice(dh * CHUNK, (dh + 1) * CHUNK)
                for kc in range(4):
                    sl = slice(kc * (KH // 4), (kc + 1) * (KH // 4))
                    nc.sync.dma_start(wph_sb[:, sl, dsl], wph_v[:, sl, dsl])
                for kc in range(4):
                    sl = slice(kc * (KH // 4), (kc + 1) * (KH // 4))
                    nc.sync.dma_start(wpl_sb[:, sl, dsl], wpl_v[:, sl, dsl])
            g_sb = const.tile([P, C // P], f32)
            nc.sync.dma_start(g_sb[:], g.ap())


            # PE p-state warmup: a dummy-matmul burst keeps the PE busy from
            # t~0 (a continuous busy streak is required to reach the 2.4GHz
            # rate) until the first x/wfc DMAs land.
            warm_sb = const.tile([P, P], f16)
            nc.vector.memset(warm_sb[:], 0.0)
            warm_ps = ps2pool.tile([P, P], f32, tag="ps2", name="warm_ps")

            def warm(n):
                for _ in range(n):
                    nc.tensor.matmul(warm_ps[:], warm_sb[:], warm_sb[:],
                                     start=True, stop=True)

            warm(30)

            tok_starts = []
            t0_ = 0
            for S_ in chunks:
                tok_starts.append(t0_)
                t0_ += S_
            a_tiles = {}

            def ensure_x(c):
                if c not in x_tiles:
                    S_ = chunks[c]
                    t0c = tok_starts[c]
                    xht = xpool.tile([P, KD, S_], f8, tag="xh", name=f"xh{c}")
                    xlt = xpool.tile([P, KD, S_], f8, tag="xl", name=f"xl{c}")
                    nc.sync.dma_start(xht[:], xh_v[:, :, t0c:t0c + S_])
                    nc.sync.dma_start(xlt[:], xl_v[:, :, t0c:t0c + S_])
                    x_tiles[c] = (xht, xlt)
                return x_tiles[c]

            def ensure_a(c):
                if c not in a_tiles:
                    S_ = chunks[c]
                    a_tiles[c] = (
                        apool.tile([P, KH, S_], f8, tag="ah", name=f"ah{c}"),
                        apool.tile([P, KH, S_], f8, tag="al", name=f"al{c}"),
                    )
                return a_tiles[c]

            def mm1_block(c, mh):
                """One h-block of chunk c: 3-term mm1 group + act chain."""
                S_ = chunks[c]
                xht, xlt = x_tiles[c]
                ah_t, al_t = a_tiles[c]
                ps1 = ps1pool.tile([P, S_], f32, tag="ps1", name=f"ps1_{c}_{mh}")
                hs = slice(mh * P, (mh + 1) * P)
                for term in ("hh", "hl", "lh"):
                    wsb, xt = {
                        "hh": (wfch_sb, xht),
                        "hl": (wfch_sb, xlt),
                        "lh": (wfcl_sb, xht),
                    }[term]
                    for gk in range(KD // 2):
                        nc.tensor.matmul(
                            ps1[:],
                            wsb[:, 2 * gk:2 * gk + 2, hs],
                            xt[:, 2 * gk:2 * gk + 2, :],
                            start=(term == "hh" and gk == 0),
                            stop=(term == "lh" and gk == KD // 2 - 1),
                            perf_mode=DR,
                        )
                # t = sqrt(sa)*prelu(h); t2 = t*t = sa*a; fp8 hi+lo split
                t_t = tpool.tile([P, S_], f16, tag="t", name=f"t_{c}_{mh}")
                nc.scalar.activation(t_t[:], ps1[:], AF.Prelu, scale=S0, alpha=0.5)
                t2_t = t2pool.tile([P, S_], f16, tag="t2", name=f"t2_{c}_{mh}")
                nc.vector.tensor_tensor(t2_t[:], t_t[:], t_t[:], mybir.AluOpType.mult)
                nc.vector.tensor_copy(ah_t[:, mh, :], t2_t[:])
                nc.gpsimd.tensor_tensor(
                    al_t[:, mh, :], t2_t[:], ah_t[:, mh, :], mybir.AluOpType.subtract
                )

            prefetched = set()
            for c, S in enumerate(chunks):
                tok0 = tok_starts[c]
                ensure_x(c)
                ah_t, al_t = ensure_a(c)
                for mh in range(1 if c in prefetched else 0, KH):
                    mm1_block(c, mh)
                # Prefetch the next chunk's first h-block ahead of mm2: it
                # covers the short semaphore stall at the mm1->mm2 boundary
                # (its inputs are resident; the aliased a-buffer write is
                # ordered after mm2's reads by the tile deps).
                if c + 1 < len(chunks):
                    ensure_x(c + 1)
                    ensure_a(c + 1)
                    mm1_block(c + 1, 0)
                    prefetched.add(c + 1)
                for dn in range(DN):
                    for ti in range(S // P):
                        gcol = tok0 // P + ti
                        ts = slice(ti * P, (ti + 1) * P)
                        # The very last group runs as two column-half psum
                        # groups so its evict->DMA trail overlaps the second
                        # half's matmuls (shorter end-of-kernel drain).
                        last = (c == len(chunks) - 1 and dn == DN - 1
                                and ti == S // P - 1)
                        # Correction sweeps partially skipped to spend spare
                        # error budget on speed: a-residual dropped on 4 of 16
                        # k-groups and Wproj-residual on 1 of 16, for a
                        # measured ~1.65e-2 rel err (vs 2e-2 budget) and 5
                        # fewer matmuls per psum group.
                        terms2 = (
                            [(ah_t, wph_sb, g) for g in range(KH // 2)]
                            + [(al_t, wph_sb, g) for g in range(KH // 2)
                               if g % 4 != 3]
                            + [(ah_t, wpl_sb, g) for g in range(KH // 2)
                               if g != 5]
                        )
                        for half in range(4 if last else 1):
                            hw_ = CHUNK // 4 if last else CHUNK
                            ds = slice(dn * CHUNK + half * hw_,
                                       dn * CHUNK + half * hw_ + hw_)
                            ps2 = ps2pool.tile([P, hw_], f32, tag="ps2",
                                               name=f"ps2_{c}_{dn}_{ti}_{half}")
                            for i, (at, wsb, gk) in enumerate(terms2):
                                nc.tensor.matmul(
                                    ps2[:],
                                    at[:, 2 * gk:2 * gk + 2, ts],
                                    wsb[:, 2 * gk:2 * gk + 2, ds],
                                    start=(i == 0),
                                    stop=(i == len(terms2) - 1),
                                    perf_mode=DR,
                                )
                            o_tile = opool.tile([P, hw_], f32, tag="ot",
                                                name=f"ot_{c}_{dn}_{ti}_{half}")
                            # fused gate+descale: out = psum * g'[token]
                            nc.scalar.activation(
                                o_tile[:], ps2[:], AF.Copy,
                                scale=g_sb[:, gcol:gcol + 1],
                            )
                            nc.sync.dma_start(out_v[:, gcol, ds], o_tile[:])
    nc.compile()
    _NC_CACHE[C] = nc
    return nc


def _route(xf, Wg):
    """Exact top-2 gating in fp32, mirroring the reference math."""
    logits = xf @ Wg.T                                   # [N, E]
    top2 = np.argpartition(logits, E - 2, axis=1)[:, E - 2:]   # [N, 2] unordered
    vals = np.take_along_axis(logits, top2, axis=1)
    m = vals.max(axis=1, keepdims=True)
    ex = np.exp(vals - m)
    w = ex / ex.sum(axis=1, keepdims=True)               # [N, 2] softmax over top-2
    return top2, w


def _split8(a, s):
    """Quantize a*s to fp8 e4m3 hi + lo residual at the same scale."""
    import ml_dtypes
    f8 = ml_dtypes.float8_e4m3
    scaled = (a * s).astype(np.float32)
    hi = scaled.astype(f8)
    lo = (scaled - hi.astype(np.float32)).astype(f8)
    return hi, lo


def run_moe(x, Wg, Wfc, Wproj, trace=False):
    from concourse import bass_utils
    import ml_dtypes

    f8 = ml_dtypes.float8_e4m3
    xf = np.ascontiguousarray(x.reshape(-1, D), dtype=np.float32)
    top2, w = _route(xf, Wg.astype(np.float32))

    toks, gates = [], []
    for e in range(E):
        sel = np.nonzero((top2 == e).any(axis=1))[0]
        ge = (w[sel] * (top2[sel] == e)).sum(axis=1).astype(np.float32)
        toks.append(sel)
        gates.append(ge)

    maxc = max(len(t) for t in toks)
    C = max(P, ((maxc + P - 1) // P) * P)

    nc = _build_nc(C)

    in_maps = []
    for e in range(E):
        te = toks[e]
        xT_e = np.zeros((D, C), np.float32)
        xT_e[:, :len(te)] = xf[te].T * SX
        xh = xT_e.astype(f8)
        xl = (xT_e - xh.astype(np.float32)).astype(f8)
        wfh, wfl = _split8(Wfc[e].T, SW)     # [D, H]
        # H-slice-major layout [H//512, D, 512] for >=512B DMA runs
        wfh = wfh.reshape(D, H // 512, 512).transpose(1, 0, 2)
        wfl = wfl.reshape(D, H // 512, 512).transpose(1, 0, 2)
        wph, wpl = _split8(Wproj[e].T, SP)   # [H, D]
        g_e = np.zeros((C,), np.float32)
        g_e[:len(te)] = gates[e] / (SA * SP)
        g_mat = np.ascontiguousarray(g_e.reshape(C // P, P).T)
        in_maps.append({
            "xh": xh, "xl": xl,
            "wfch": np.ascontiguousarray(wfh), "wfcl": np.ascontiguousarray(wfl),
            "wph": np.ascontiguousarray(wph), "wpl": np.ascontiguousarray(wpl),
            "g": g_mat,
        })

    res = bass_utils.run_bass_kernel_spmd(
        nc, in_maps, core_ids=list(range(E)), trace=False
    )

    out = np.zeros((N, D), np.float32)
    for e in range(E):
        te = toks[e]
        out[te] += res.results[e]["outp"][:len(te)]
    return out.reshape(B, T, D), res


def kernel(x, Wg, Wfc, Wproj):
    out, _ = run_moe(np.asarray(x), np.asarray(Wg), np.asarray(Wfc), np.asarray(Wproj))
    return out


# revision 45
# speedup vs baseline: 1.0001x; 1.0001x over previous
"""MoE MLP (top-2 of 8 experts) Trainium2 kernel — fp8 DoubleRow edition.

Strategy: expert-parallel across the 8 NeuronCores (host computes exact top-2
gating, gathers each expert's tokens; core e runs expert e's two matmuls).

The two big matmuls run on the PE in fp8-e4m3 DoubleRow mode (2 contraction
subtiles per instruction at 0.5 cycles/row = 4x the fp16 MAC rate). Plain fp8
is too coarse (~5e-2 rel err), so every operand is split into hi + lo fp8
parts at a SHARED power-of-2 scale (the residual lands partly in fp8
denormals, which the PE honors) and each 256-wide contraction group runs
three DoubleRow matmuls accumulating into one PSUM group:

    x@W ~= xh@Wh + xl@Wh + xh@Wl        (the lo*lo term is negligible)

mm2 additionally skips the a-residual (al@Wh) correction on 4 of its 16
contraction groups, trading measured rel err up to ~1.4e-2 (vs the 2e-2
budget) for 4 fewer matmuls per output tile. Net matmul cycle count is
0.72x fp16 -> ~1.4x over the fp16 PE roofline.

Per-core dataflow (contraction dim on SBUF partitions throughout):
  mm1: psum[h_blk(128), tok(S)] += 3-term DR over wfc{h,l}[d,2,h] x{h,l}[d,2,S]
  act: t = prelu(psum * 2^-14, 0.5) in fp16 (ScalarE)   [t = sqrt(sa)*prelu(h)]
       t2 = t*t fp16 (VectorE)                          [t2 = sa*a]
       ah = cast_fp8(t2) (ScalarE); al = t2 - ah -> fp8 (VectorE)
  mm2: psum[tok(128), d(512)] += 3-term DR over a{h,l}[h,2,tok] wproj{h,l}[h,2,d]
  evict: out = psum * g'[token]  (g' = gate/(sa*sp), per-partition scale)

Host scatters per-expert rows back and sums (each token in exactly 2 lists).
"""

import numpy as np
from contextlib import ExitStack

B, T, D, H, E = 4, 2048, 1024, 4096, 8
N = B * T
P = 128
CHUNK = 512

# power-of-2 quantization scales (shared between hi and lo parts)
SX = 32.0        # x:    |x|max ~5.3  -> 169 < 240
SW = 1024.0      # Wfc:  |W|max ~0.11 -> 108
SA = 4.0         # a:    |a|max ~15   -> 58
SP = 1024.0      # Wproj
S0 = float(np.sqrt(SA) / (SX * SW))   # prelu pre-scale = 2/(32*1024) = 2^-14

_NC_CACHE = {}


def _build_nc(C):
    """Build + compile the per-core Bass program for capacity C tokens."""
    if C in _NC_CACHE:
        return _NC_CACHE[C]
    import concourse.bacc as bacc
    import concourse.tile as tile
    import concourse.mybir as mybir

    assert C % P == 0
    f8 = mybir.dt.float8e4
    f16 = mybir.dt.float16
    f32 = mybir.dt.float32
    AF = mybir.ActivationFunctionType
    DR = mybir.MatmulPerfMode.DoubleRow

    WS = 512             # wfc DRAM H-tile width: keeps DMA runs >= 512B
    NSL = H // WS        # 8 slices

    nc = bacc.Bacc(None, target_bir_lowering=False, debug=False)
    xh_d = nc.dram_tensor("xh", [D, C], f8, kind="ExternalInput")
    xl_d = nc.dram_tensor("xl", [D, C], f8, kind="ExternalInput")
    # wfc stored H-slice-major so each [D, 512] slice is 512B-contiguous
    wfch_d = nc.dram_tensor("wfch", [NSL, D, WS], f8, kind="ExternalInput")
    wfcl_d = nc.dram_tensor("wfcl", [NSL, D, WS], f8, kind="ExternalInput")
    wph_d = nc.dram_tensor("wph", [H, D], f8, kind="ExternalInput")
    wpl_d = nc.dram_tensor("wpl", [H, D], f8, kind="ExternalInput")
    g = nc.dram_tensor("g", [P, C // P], f32, kind="ExternalInput")
    out = nc.dram_tensor("outp", [C, D], f32, kind="ExternalOutput")

    xh_v = xh_d.ap().rearrange("(ko p) c -> p ko c", p=P)      # [128, 8, C]
    xl_v = xl_d.ap().rearrange("(ko p) c -> p ko c", p=P)
    wfch_v = wfch_d.ap().rearrange("s (ko p) j -> p s ko j", p=P)  # [128, 8, 8, 512]
    wfcl_v = wfcl_d.ap().rearrange("s (ko p) j -> p s ko j", p=P)
    wph_v = wph_d.ap().rearrange("(ko p) d -> p ko d", p=P)    # [128, 32, D]
    wpl_v = wpl_d.ap().rearrange("(ko p) d -> p ko d", p=P)
    out_v = out.ap().rearrange("(c p) d -> p c d", p=P)        # [128, C//128, D]

    KD = D // P          # 8  k-subtiles for mm1 (4 DoubleRow groups)
    KH = H // P          # 32 k-subtiles for mm2 (16 DoubleRow groups)
    DN = D // CHUNK      # 2 output-column blocks

    # Tail chunk goes SECOND: chunk 0 must be full-size (its mm1 is the DMA
    # window for the wproj load), and ending on a full chunk hides the tail's
    # short act-chain lag and shrinks the end-of-kernel drain.
    chunks = [CHUNK] * (C // CHUNK)
    if C % CHUNK:
        chunks.insert(1, C % CHUNK)

    with tile.TileContext(nc) as tc:
        with ExitStack() as ctx:
            const = ctx.enter_context(tc.tile_pool(name="const", bufs=1))
            xpool = ctx.enter_context(tc.tile_pool(name="xp", bufs=2))
            apool = ctx.enter_context(tc.tile_pool(name="apool", bufs=1))
            tpool = ctx.enter_context(tc.tile_pool(name="tp", bufs=3))
            t2pool = ctx.enter_context(tc.tile_pool(name="t2p", bufs=3))
            opool = ctx.enter_context(tc.tile_pool(name="op", bufs=4))
            ps1pool = ctx.enter_context(tc.tile_pool(name="ps1", bufs=4, space="PSUM"))
            ps2pool = ctx.enter_context(tc.tile_pool(name="ps2", bufs=4, space="PSUM"))

            # Startup-critical DMAs first. mm1 group mh consumes wfc H-slice
            # mh//4; slice 0 + x chunk0 gate the first matmuls.
            x_tiles = {}
            xht = xpool.tile([P, KD, chunks[0]], f8, tag="xh", name="xh0")
            xlt = xpool.tile([P, KD, chunks[0]], f8, tag="xl", name="xl0")
            x_tiles[0] = (xht, xlt)
            wfch_sb = const.tile([P, KD, H], f8)
            wfcl_sb = const.tile([P, KD, H], f8)

            def wfc_slice(si):
                ws = slice(si * WS, (si + 1) * WS)
                nc.sync.dma_start(wfch_sb[:, :, ws], wfch_v[:, si, :, :])
                nc.sync.dma_start(wfcl_sb[:, :, ws], wfcl_v[:, si, :, :])

            ws0 = slice(0, WS)
            nc.sync.dma_start(wfch_sb[:, :, ws0], wfch_v[:, 0, :, :])
            nc.sync.dma_start(xht[:, 0:4, :], xh_v[:, 0:4, 0:chunks[0]])
            nc.sync.dma_start(xht[:, 4:KD, :], xh_v[:, 4:KD, 0:chunks[0]])
            nc.sync.dma_start(xlt[:], xl_v[:, :, 0:chunks[0]])
            nc.sync.dma_start(wfcl_sb[:, :, ws0], wfcl_v[:, 0, :, :])
            for si in range(1, NSL):
                wfc_slice(si)
            # wproj needed only when mm2 of chunk 0 starts (~48us in), and
            # with dn as the outer mm2 loop the second D-half isn't touched
            # until ~20us later still — load half-D at a time.
            wph_sb = const.tile([P, KH, D], f8)
            wpl_sb = const.tile([P, KH, D], f8)
            for dh in range(DN):
                dsl = slice(dh * CHUNK, (dh + 1) * CHUNK)
                for kc in range(4):
                    sl = slice(kc * (KH // 4), (kc + 1) * (KH // 4))
                    nc.sync.dma_start(wph_sb[:, sl, dsl], wph_v[:, sl, dsl])
                for kc in range(4):
                    sl = slice(kc * (KH // 4), (kc + 1) * (KH // 4))
                    nc.sync.dma_start(wpl_sb[:, sl, dsl], wpl_v[:, sl, dsl])
            g_sb = const.tile([P, C // P], f32)
            nc.sync.dma_start(g_sb[:], g.ap())


            # PE p-state warmup: a dummy-matmul burst keeps the PE busy from
            # t~0 (a continuous busy streak is required to reach the 2.4GHz
            # rate) until the first x/wfc DMAs land.
            warm_sb = const.tile([P, P], f16)
            nc.vector.memset(warm_sb[:], 0.0)
            warm_ps = ps2pool.tile([P, P], f32, tag="ps2", name="warm_ps")

            def warm(n):
                for _ in range(n):
                    nc.tensor.matmul(warm_ps[:], warm_sb[:], warm_sb[:],
                                     start=True, stop=True)

            warm(30)

            tok_starts = []
            t0_ = 0
            for S_ in chunks:
                tok_starts.append(t0_)
                t0_ += S_
            a_tiles = {}

            def ensure_x(c):
                if c not in x_tiles:
                    S_ = chunks[c]
                    t0c = tok_starts[c]
                    xht = xpool.tile([P, KD, S_], f8, tag="xh", name=f"xh{c}")
                    xlt = xpool.tile([P, KD, S_], f8, tag="xl", name=f"xl{c}")
                    nc.sync.dma_start(xht[:], xh_v[:, :, t0c:t0c + S_])
                    nc.sync.dma_start(xlt[:], xl_v[:, :, t0c:t0c + S_])
                    x_tiles[c] = (xht, xlt)
                return x_tiles[c]

            def ensure_a(c):
                if c not in a_tiles:
                    S_ = chunks[c]
                    a_tiles[c] = (
                        apool.tile([P, KH, S_], f8, tag="ah", name=f"ah{c}"),
                        apool.tile([P, KH, S_], f8, tag="al", name=f"al{c}"),
                    )
                return a_tiles[c]

            def mm1_block(c, mh):
                """One h-block of chunk c: 3-term mm1 group + act chain."""
                S_ = chunks[c]
                xht, xlt = x_tiles[c]
                ah_t, al_t = a_tiles[c]
                ps1 = ps1pool.tile([P, S_], f32, tag="ps1", name=f"ps1_{c}_{mh}")
                hs = slice(mh * P, (mh + 1) * P)
                for term in ("hh", "hl", "lh"):
                    wsb, xt = {
                        "hh": (wfch_sb, xht),
                        "hl": (wfch_sb, xlt),
                        "lh": (wfcl_sb, xht),
                    }[term]
                    for gk in range(KD // 2):
                        nc.tensor.matmul(
                            ps1[:],
                            wsb[:, 2 * gk:2 * gk + 2, hs],
                            xt[:, 2 * gk:2 * gk + 2, :],
                            start=(term == "hh" and gk == 0),
                            stop=(term == "lh" and gk == KD // 2 - 1),
                            perf_mode=DR,
                        )
                # t = sqrt(sa)*prelu(h); t2 = t*t = sa*a; fp8 hi+lo split
                t_t = tpool.tile([P, S_], f16, tag="t", name=f"t_{c}_{mh}")
                nc.scalar.activation(t_t[:], ps1[:], AF.Prelu, scale=S0, alpha=0.5)
                t2_t = t2pool.tile([P, S_], f16, tag="t2", name=f"t2_{c}_{mh}")
                nc.vector.tensor_tensor(t2_t[:], t_t[:], t_t[:], mybir.AluOpType.mult)
                nc.vector.tensor_copy(ah_t[:, mh, :], t2_t[:])
                nc.gpsimd.tensor_tensor(
                    al_t[:, mh, :], t2_t[:], ah_t[:, mh, :], mybir.AluOpType.subtract
                )

            pre_blocks = {}
            for c, S in enumerate(chunks):
                tok0 = tok_starts[c]
                ensure_x(c)
                ah_t, al_t = ensure_a(c)
                for mh in range(pre_blocks.get(c, 0), KH):
                    mm1_block(c, mh)
                # Prefetch the next chunk's first h-block(s) ahead of mm2:
                # they cover the cross-engine dependency join at the mm1->mm2
                # boundary (inputs are resident; the aliased a-buffer writes
                # are ordered after mm2's reads by the tile deps). A narrow
                # tail block is ~320ns of PE work, so prefetch two of those.
                if c + 1 < len(chunks):
                    ensure_x(c + 1)
                    ensure_a(c + 1)
                    npre = 1 if chunks[c + 1] >= CHUNK else 2
                    for mh in range(npre):
                        mm1_block(c + 1, mh)
                    pre_blocks[c + 1] = npre
                for dn in range(DN):
                    for ti in range(S // P):
                        gcol = tok0 // P + ti
                        ts = slice(ti * P, (ti + 1) * P)
                        # The very last group runs as two column-half psum
                        # groups so its evict->DMA trail overlaps the second
                        # half's matmuls (shorter end-of-kernel drain).
                        last = (c == len(chunks) - 1 and dn == DN - 1
                                and ti == S // P - 1)
                        # Correction sweeps partially skipped to spend spare
                        # error budget on speed: a-residual dropped on 4 of 16
                        # k-groups and Wproj-residual on 1 of 16, for a
                        # measured ~1.65e-2 rel err (vs 2e-2 budget) and 5
                        # fewer matmuls per psum group.
                        terms2 = (
                            [(ah_t, wph_sb, g) for g in range(KH // 2)]
                            + [(al_t, wph_sb, g) for g in range(KH // 2)
                               if g % 4 != 3]
                            + [(ah_t, wpl_sb, g) for g in range(KH // 2)
                               if g != 5]
                        )
                        for half in range(4 if last else 1):
                            hw_ = CHUNK // 4 if last else CHUNK
                            ds = slice(dn * CHUNK + half * hw_,
                                       dn * CHUNK + half * hw_ + hw_)
                            ps2 = ps2pool.tile([P, hw_], f32, tag="ps2",
                                               name=f"ps2_{c}_{dn}_{ti}_{half}")
                            for i, (at, wsb, gk) in enumerate(terms2):
                                nc.tensor.matmul(
                                    ps2[:],
                                    at[:, 2 * gk:2 * gk + 2, ts],
                                    wsb[:, 2 * gk:2 * gk + 2, ds],
                                    start=(i == 0),
                                    stop=(i == len(terms2) - 1),
                                    perf_mode=DR,
                                )
                            o_tile = opool.tile([P, hw_], f32, tag="ot",
                                                name=f"ot_{c}_{dn}_{ti}_{half}")
                            # fused gate+descale: out = psum * g'[token]
                            nc.scalar.activation(
                                o_tile[:], ps2[:], AF.Copy,
                                scale=g_sb[:, gcol:gcol + 1],
                            )
                            nc.sync.dma_start(out_v[:, gcol, ds], o_tile[:])
    nc.compile()
    _NC_CACHE[C] = nc
    return nc


def _route(xf, Wg):
    """Exact top-2 gating in fp32, mirroring the reference math."""
    logits = xf @ Wg.T                                   # [N, E]
    top2 = np.argpartition(logits, E - 2, axis=1)[:, E - 2:]   # [N, 2] unordered
    vals = np.take_along_axis(logits, top2, axis=1)
    m = vals.max(axis=1, keepdims=True)
    ex = np.exp(vals - m)
    w = ex / ex.sum(axis=1, keepdims=True)               # [N, 2] softmax over top-2
    return top2, w


def _split8(a, s):
    """Quantize a*s to fp8 e4m3 hi + lo residual at the same scale."""
    import ml_dtypes
    f8 = ml_dtypes.float8_e4m3
    scaled = (a * s).astype(np.float32)
    hi = scaled.astype(f8)
    lo = (scaled - hi.astype(np.float32)).astype(f8)
    return hi, lo


def run_moe(x, Wg, Wfc, Wproj, trace=False):
    from concourse import bass_utils
    import ml_dtypes

    f8 = ml_dtypes.float8_e4m3
    xf = np.ascontiguousarray(x.reshape(-1, D), dtype=np.float32)
    top2, w = _route(xf, Wg.astype(np.float32))

    toks, gates = [], []
    for e in range(E):
        sel = np.nonzero((top2 == e).any(axis=1))[0]
        ge = (w[sel] * (top2[sel] == e)).sum(axis=1).astype(np.float32)
        toks.append(sel)
        gates.append(ge)

    maxc = max(len(t) for t in toks)
    C = max(P, ((maxc + P - 1) // P) * P)

    nc = _build_nc(C)

    in_maps = []
    for e in range(E):
        te = toks[e]
        xT_e = np.zeros((D, C), np.float32)
        xT_e[:, :len(te)] = xf[te].T * SX
        xh = xT_e.astype(f8)
        xl = (xT_e - xh.astype(np.float32)).astype(f8)
        wfh, wfl = _split8(Wfc[e].T, SW)     # [D, H]
        # H-slice-major layout [H//512, D, 512] for >=512B DMA runs
        wfh = wfh.reshape(D, H // 512, 512).transpose(1, 0, 2)
        wfl = wfl.reshape(D, H // 512, 512).transpose(1, 0, 2)
        wph, wpl = _split8(Wproj[e].T, SP)   # [H, D]
        g_e = np.zeros((C,), np.float32)
        g_e[:len(te)] = gates[e] / (SA * SP)
        g_mat = np.ascontiguousarray(g_e.reshape(C // P, P).T)
        in_maps.append({
            "xh": xh, "xl": xl,
            "wfch": np.ascontiguousarray(wfh), "wfcl": np.ascontiguousarray(wfl),
            "wph": np.ascontiguousarray(wph), "wpl": np.ascontiguousarray(wpl),
            "g": g_mat,
        })

    res = bass_utils.run_bass_kernel_spmd(
        nc, in_maps, core_ids=list(range(E)), trace=False
    )

    out = np.zeros((N, D), np.float32)
    for e in range(E):
        te = toks[e]
        out[te] += res.results[e]["outp"][:len(te)]
    return out.reshape(B, T, D), res


def kernel(x, Wg, Wfc, Wproj):
    out, _ = run_moe(np.asarray(x), np.asarray(Wg), np.asarray(Wfc), np.asarray(Wproj))
    return out


# revision 46
# speedup vs baseline: 1.0049x; 1.0048x over previous
"""MoE MLP (top-2 of 8 experts) Trainium2 kernel — fp8 DoubleRow edition.

Strategy: expert-parallel across the 8 NeuronCores (host computes exact top-2
gating, gathers each expert's tokens; core e runs expert e's two matmuls).

The two big matmuls run on the PE in fp8-e4m3 DoubleRow mode (2 contraction
subtiles per instruction at 0.5 cycles/row = 4x the fp16 MAC rate). Plain fp8
is too coarse (~5e-2 rel err), so every operand is split into hi + lo fp8
parts at a SHARED power-of-2 scale (the residual lands partly in fp8
denormals, which the PE honors) and each 256-wide contraction group runs
three DoubleRow matmuls accumulating into one PSUM group:

    x@W ~= xh@Wh + xl@Wh + xh@Wl        (the lo*lo term is negligible)

mm2 additionally skips the a-residual (al@Wh) correction on 4 of its 16
contraction groups, trading measured rel err up to ~1.4e-2 (vs the 2e-2
budget) for 4 fewer matmuls per output tile. Net matmul cycle count is
0.72x fp16 -> ~1.4x over the fp16 PE roofline.

Per-core dataflow (contraction dim on SBUF partitions throughout):
  mm1: psum[h_blk(128), tok(S)] += 3-term DR over wfc{h,l}[d,2,h] x{h,l}[d,2,S]
  act: t = prelu(psum * 2^-14, 0.5) in fp16 (ScalarE)   [t = sqrt(sa)*prelu(h)]
       t2 = t*t fp16 (VectorE)                          [t2 = sa*a]
       ah = cast_fp8(t2) (ScalarE); al = t2 - ah -> fp8 (VectorE)
  mm2: psum[tok(128), d(512)] += 3-term DR over a{h,l}[h,2,tok] wproj{h,l}[h,2,d]
  evict: out = psum * g'[token]  (g' = gate/(sa*sp), per-partition scale)

Host scatters per-expert rows back and sums (each token in exactly 2 lists).
"""

import numpy as np
from contextlib import ExitStack

B, T, D, H, E = 4, 2048, 1024, 4096, 8
N = B * T
P = 128
CHUNK = 512

# power-of-2 quantization scales (shared between hi and lo parts)
SX = 32.0        # x:    |x|max ~5.3  -> 169 < 240
SW = 1024.0      # Wfc:  |W|max ~0.11 -> 108
SA = 4.0         # a:    |a|max ~15   -> 58
SP = 1024.0      # Wproj
S0 = float(np.sqrt(SA) / (SX * SW))   # prelu pre-scale = 2/(32*1024) = 2^-14

_NC_CACHE = {}


def _build_nc(C):
    """Build + compile the per-core Bass program for capacity C tokens."""
    if C in _NC_CACHE:
        return _NC_CACHE[C]
    import concourse.bacc as bacc
    import concourse.tile as tile
    import concourse.mybir as mybir

    assert C % P == 0
    f8 = mybir.dt.float8e4
    f16 = mybir.dt.float16
    f32 = mybir.dt.float32
    AF = mybir.ActivationFunctionType
    DR = mybir.MatmulPerfMode.DoubleRow

    WS = 512             # wfc DRAM H-tile width: keeps DMA runs >= 512B
    NSL = H // WS        # 8 slices

    nc = bacc.Bacc(None, target_bir_lowering=False, debug=False)
    xh_d = nc.dram_tensor("xh", [D, C], f8, kind="ExternalInput")
    xl_d = nc.dram_tensor("xl", [D, C], f8, kind="ExternalInput")
    # wfc stored H-slice-major so each [D, 512] slice is 512B-contiguous
    wfch_d = nc.dram_tensor("wfch", [NSL, D, WS], f8, kind="ExternalInput")
    wfcl_d = nc.dram_tensor("wfcl", [NSL, D, WS], f8, kind="ExternalInput")
    wph_d = nc.dram_tensor("wph", [H, D], f8, kind="ExternalInput")
    wpl_d = nc.dram_tensor("wpl", [H, D], f8, kind="ExternalInput")
    g = nc.dram_tensor("g", [P, C // P], f32, kind="ExternalInput")
    out = nc.dram_tensor("outp", [C, D], f32, kind="ExternalOutput")

    xh_v = xh_d.ap().rearrange("(ko p) c -> p ko c", p=P)      # [128, 8, C]
    xl_v = xl_d.ap().rearrange("(ko p) c -> p ko c", p=P)
    wfch_v = wfch_d.ap().rearrange("s (ko p) j -> p s ko j", p=P)  # [128, 8, 8, 512]
    wfcl_v = wfcl_d.ap().rearrange("s (ko p) j -> p s ko j", p=P)
    wph_v = wph_d.ap().rearrange("(ko p) d -> p ko d", p=P)    # [128, 32, D]
    wpl_v = wpl_d.ap().rearrange("(ko p) d -> p ko d", p=P)
    out_v = out.ap().rearrange("(c p) d -> p c d", p=P)        # [128, C//128, D]

    KD = D // P          # 8  k-subtiles for mm1 (4 DoubleRow groups)
    KH = H // P          # 32 k-subtiles for mm2 (16 DoubleRow groups)
    DN = D // CHUNK      # 2 output-column blocks

    # Tail chunk goes SECOND: chunk 0 must be full-size (its mm1 is the DMA
    # window for the wproj load), and ending on a full chunk hides the tail's
    # short act-chain lag and shrinks the end-of-kernel drain.
    chunks = [CHUNK] * (C // CHUNK)
    if C % CHUNK:
        chunks.insert(1, C % CHUNK)

    with tile.TileContext(nc) as tc:
        with ExitStack() as ctx:
            const = ctx.enter_context(tc.tile_pool(name="const", bufs=1))
            xpool = ctx.enter_context(tc.tile_pool(name="xp", bufs=2))
            apool = ctx.enter_context(tc.tile_pool(name="apool", bufs=1))
            tpool = ctx.enter_context(tc.tile_pool(name="tp", bufs=3))
            t2pool = ctx.enter_context(tc.tile_pool(name="t2p", bufs=3))
            opool = ctx.enter_context(tc.tile_pool(name="op", bufs=4))
            ps1pool = ctx.enter_context(tc.tile_pool(name="ps1", bufs=4, space="PSUM"))
            ps2pool = ctx.enter_context(tc.tile_pool(name="ps2", bufs=4, space="PSUM"))

            # Startup-critical DMAs first. mm1 group mh consumes wfc H-slice
            # mh//4; slice 0 + x chunk0 gate the first matmuls.
            x_tiles = {}
            xht = xpool.tile([P, KD, chunks[0]], f8, tag="xh", name="xh0")
            xlt = xpool.tile([P, KD, chunks[0]], f8, tag="xl", name="xl0")
            x_tiles[0] = (xht, xlt)
            wfch_sb = const.tile([P, KD, H], f8)
            wfcl_sb = const.tile([P, KD, H], f8)

            def wfc_slice(si):
                ws = slice(si * WS, (si + 1) * WS)
                nc.sync.dma_start(wfch_sb[:, :, ws], wfch_v[:, si, :, :])
                nc.sync.dma_start(wfcl_sb[:, :, ws], wfcl_v[:, si, :, :])

            ws0 = slice(0, WS)
            nc.sync.dma_start(wfch_sb[:, :, ws0], wfch_v[:, 0, :, :])
            nc.sync.dma_start(xht[:, 0:4, :], xh_v[:, 0:4, 0:chunks[0]])
            nc.sync.dma_start(xht[:, 4:KD, :], xh_v[:, 4:KD, 0:chunks[0]])
            nc.sync.dma_start(xlt[:], xl_v[:, :, 0:chunks[0]])
            nc.sync.dma_start(wfcl_sb[:, :, ws0], wfcl_v[:, 0, :, :])
            for si in range(1, NSL):
                wfc_slice(si)
            # wproj needed only when mm2 of chunk 0 starts (~48us in), and
            # with dn as the outer mm2 loop the second D-half isn't touched
            # until ~20us later still — load half-D at a time.
            wph_sb = const.tile([P, KH, D], f8)
            wpl_sb = const.tile([P, KH, D], f8)
            for dh in range(DN):
                dsl = slice(dh * CHUNK, (dh + 1) * CHUNK)
                for kc in range(4):
                    sl = slice(kc * (KH // 4), (kc + 1) * (KH // 4))
                    nc.sync.dma_start(wph_sb[:, sl, dsl], wph_v[:, sl, dsl])
                for kc in range(4):
                    sl = slice(kc * (KH // 4), (kc + 1) * (KH // 4))
                    nc.sync.dma_start(wpl_sb[:, sl, dsl], wpl_v[:, sl, dsl])
            g_sb = const.tile([P, C // P], f32)
            nc.sync.dma_start(g_sb[:], g.ap())


            # PE p-state warmup: a dummy-matmul burst keeps the PE busy from
            # t~0 (a continuous busy streak is required to reach the 2.4GHz
            # rate) until the first x/wfc DMAs land.
            warm_sb = const.tile([P, P], f16)
            nc.vector.memset(warm_sb[:], 0.0)
            warm_ps = ps2pool.tile([P, P], f32, tag="ps2", name="warm_ps")

            def warm(n):
                for _ in range(n):
                    nc.tensor.matmul(warm_ps[:], warm_sb[:], warm_sb[:],
                                     start=True, stop=True)

            warm(30)

            tok_starts = []
            t0_ = 0
            for S_ in chunks:
                tok_starts.append(t0_)
                t0_ += S_
            a_tiles = {}

            def ensure_x(c):
                if c not in x_tiles:
                    S_ = chunks[c]
                    t0c = tok_starts[c]
                    xht = xpool.tile([P, KD, S_], f8, tag="xh", name=f"xh{c}")
                    xlt = xpool.tile([P, KD, S_], f8, tag="xl", name=f"xl{c}")
                    nc.sync.dma_start(xht[:], xh_v[:, :, t0c:t0c + S_])
                    nc.sync.dma_start(xlt[:], xl_v[:, :, t0c:t0c + S_])
                    x_tiles[c] = (xht, xlt)
                return x_tiles[c]

            def ensure_a(c):
                if c not in a_tiles:
                    S_ = chunks[c]
                    a_tiles[c] = (
                        apool.tile([P, KH, S_], f8, tag="ah", name=f"ah{c}"),
                        apool.tile([P, KH, S_], f8, tag="al", name=f"al{c}"),
                    )
                return a_tiles[c]

            def mm1_block(c, mh):
                """One h-block of chunk c: 3-term mm1 group + act chain."""
                S_ = chunks[c]
                xht, xlt = x_tiles[c]
                ah_t, al_t = a_tiles[c]
                ps1 = ps1pool.tile([P, S_], f32, tag="ps1", name=f"ps1_{c}_{mh}")
                hs = slice(mh * P, (mh + 1) * P)
                # Wfc-residual sweep skipped for one h-block (mh 16): part of
                # the measured ~1.74e-2 error-for-speed trade vs the 2e-2 gate.
                terms = ["hh", "hl"] + ([] if mh == 16 else ["lh"])
                for term in terms:
                    wsb, xt = {
                        "hh": (wfch_sb, xht),
                        "hl": (wfch_sb, xlt),
                        "lh": (wfcl_sb, xht),
                    }[term]
                    for gk in range(KD // 2):
                        nc.tensor.matmul(
                            ps1[:],
                            wsb[:, 2 * gk:2 * gk + 2, hs],
                            xt[:, 2 * gk:2 * gk + 2, :],
                            start=(term == "hh" and gk == 0),
                            stop=(term == terms[-1] and gk == KD // 2 - 1),
                            perf_mode=DR,
                        )
                # t = sqrt(sa)*prelu(h); t2 = t*t = sa*a; fp8 hi+lo split
                t_t = tpool.tile([P, S_], f16, tag="t", name=f"t_{c}_{mh}")
                nc.scalar.activation(t_t[:], ps1[:], AF.Prelu, scale=S0, alpha=0.5)
                t2_t = t2pool.tile([P, S_], f16, tag="t2", name=f"t2_{c}_{mh}")
                nc.vector.tensor_tensor(t2_t[:], t_t[:], t_t[:], mybir.AluOpType.mult)
                nc.vector.tensor_copy(ah_t[:, mh, :], t2_t[:])
                nc.gpsimd.tensor_tensor(
                    al_t[:, mh, :], t2_t[:], ah_t[:, mh, :], mybir.AluOpType.subtract
                )

            pre_blocks = {}
            for c, S in enumerate(chunks):
                tok0 = tok_starts[c]
                ensure_x(c)
                ah_t, al_t = ensure_a(c)
                for mh in range(pre_blocks.get(c, 0), KH):
                    mm1_block(c, mh)
                # Prefetch the next chunk's first h-block(s) ahead of mm2:
                # they cover the cross-engine dependency join at the mm1->mm2
                # boundary (inputs are resident; the aliased a-buffer writes
                # are ordered after mm2's reads by the tile deps). A narrow
                # tail block is ~320ns of PE work, so prefetch two of those.
                if c + 1 < len(chunks):
                    ensure_x(c + 1)
                    ensure_a(c + 1)
                    npre = 1 if chunks[c + 1] >= CHUNK else 2
                    for mh in range(npre):
                        mm1_block(c + 1, mh)
                    pre_blocks[c + 1] = npre
                for dn in range(DN):
                    for ti in range(S // P):
                        gcol = tok0 // P + ti
                        ts = slice(ti * P, (ti + 1) * P)
                        # The very last group runs as two column-half psum
                        # groups so its evict->DMA trail overlaps the second
                        # half's matmuls (shorter end-of-kernel drain).
                        last = (c == len(chunks) - 1 and dn == DN - 1
                                and ti == S // P - 1)
                        # Correction sweeps partially skipped to spend spare
                        # error budget on speed: a-residual dropped on 4 of 16
                        # k-groups and Wproj-residual on 1 of 16, for a
                        # measured ~1.65e-2 rel err (vs 2e-2 budget) and 5
                        # fewer matmuls per psum group.
                        terms2 = (
                            [(ah_t, wph_sb, g) for g in range(KH // 2)]
                            + [(al_t, wph_sb, g) for g in range(KH // 2)
                               if g % 4 != 3]
                            + [(ah_t, wpl_sb, g) for g in range(KH // 2)
                               if g != 5]
                        )
                        for half in range(4 if last else 1):
                            hw_ = CHUNK // 4 if last else CHUNK
                            ds = slice(dn * CHUNK + half * hw_,
                                       dn * CHUNK + half * hw_ + hw_)
                            ps2 = ps2pool.tile([P, hw_], f32, tag="ps2",
                                               name=f"ps2_{c}_{dn}_{ti}_{half}")
                            for i, (at, wsb, gk) in enumerate(terms2):
                                nc.tensor.matmul(
                                    ps2[:],
                                    at[:, 2 * gk:2 * gk + 2, ts],
                                    wsb[:, 2 * gk:2 * gk + 2, ds],
                                    start=(i == 0),
                                    stop=(i == len(terms2) - 1),
                                    perf_mode=DR,
                                )
                            o_tile = opool.tile([P, hw_], f32, tag="ot",
                                                name=f"ot_{c}_{dn}_{ti}_{half}")
                            # fused gate+descale: out = psum * g'[token]
                            nc.scalar.activation(
                                o_tile[:], ps2[:], AF.Copy,
                                scale=g_sb[:, gcol:gcol + 1],
                            )
                            nc.sync.dma_start(out_v[:, gcol, ds], o_tile[:])
    nc.compile()
    _NC_CACHE[C] = nc
    return nc


def _route(xf, Wg):
    """Exact top-2 gating in fp32, mirroring the reference math."""
    logits = xf @ Wg.T                                   # [N, E]
    top2 = np.argpartition(logits, E - 2, axis=1)[:, E - 2:]   # [N, 2] unordered
    vals = np.take_along_axis(logits, top2, axis=1)
    m = vals.max(axis=1, keepdims=True)
    ex = np.exp(vals - m)
    w = ex / ex.sum(axis=1, keepdims=True)               # [N, 2] softmax over top-2
    return top2, w


def _split8(a, s):
    """Quantize a*s to fp8 e4m3 hi + lo residual at the same scale."""
    import ml_dtypes
    f8 = ml_dtypes.float8_e4m3
    scaled = (a * s).astype(np.float32)
    hi = scaled.astype(f8)
    lo = (scaled - hi.astype(np.float32)).astype(f8)
    return hi, lo


def run_moe(x, Wg, Wfc, Wproj, trace=False):
    from concourse import bass_utils
    import ml_dtypes

    f8 = ml_dtypes.float8_e4m3
    xf = np.ascontiguousarray(x.reshape(-1, D), dtype=np.float32)
    top2, w = _route(xf, Wg.astype(np.float32))

    toks, gates = [], []
    for e in range(E):
        sel = np.nonzero((top2 == e).any(axis=1))[0]
        ge = (w[sel] * (top2[sel] == e)).sum(axis=1).astype(np.float32)
        toks.append(sel)
        gates.append(ge)

    maxc = max(len(t) for t in toks)
    C = max(P, ((maxc + P - 1) // P) * P)

    nc = _build_nc(C)

    in_maps = []
    for e in range(E):
        te = toks[e]
        xT_e = np.zeros((D, C), np.float32)
        xT_e[:, :len(te)] = xf[te].T * SX
        xh = xT_e.astype(f8)
        xl = (xT_e - xh.astype(np.float32)).astype(f8)
        wfh, wfl = _split8(Wfc[e].T, SW)     # [D, H]
        # H-slice-major layout [H//512, D, 512] for >=512B DMA runs
        wfh = wfh.reshape(D, H // 512, 512).transpose(1, 0, 2)
        wfl = wfl.reshape(D, H // 512, 512).transpose(1, 0, 2)
        wph, wpl = _split8(Wproj[e].T, SP)   # [H, D]
        g_e = np.zeros((C,), np.float32)
        g_e[:len(te)] = gates[e] / (SA * SP)
        g_mat = np.ascontiguousarray(g_e.reshape(C // P, P).T)
        in_maps.append({
            "xh": xh, "xl": xl,
            "wfch": np.ascontiguousarray(wfh), "wfcl": np.ascontiguousarray(wfl),
            "wph": np.ascontiguousarray(wph), "wpl": np.ascontiguousarray(wpl),
            "g": g_mat,
        })

    res = bass_utils.run_bass_kernel_spmd(
        nc, in_maps, core_ids=list(range(E)), trace=False
    )

    out = np.zeros((N, D), np.float32)
    for e in range(E):
        te = toks[e]
        out[te] += res.results[e]["outp"][:len(te)]
    return out.reshape(B, T, D), res


def kernel(x, Wg, Wfc, Wproj):
    out, _ = run_moe(np.asarray(x), np.asarray(Wg), np.asarray(Wfc), np.asarray(Wproj))
    return out
